# revision 9
# baseline (speedup 1.0000x reference)
"""Trainium2 Bass kernel: pre-LN multi-head attention block (B=8, L=1024,
D=1024, H=16, dk=dv=64), data-parallel over batch across 8 NeuronCores.

Design (v2 lineage: no PE transposes, paired S matmuls):
  - q/k/v pre-cast to bf16 on host; k and v are transposed on the HOST, so
    the kernel DMAs kT/vT directly and the PE never runs a transpose for
    them. q stays token-major (residual + LN stats); qn is LN'd on ACT and
    transposed by the DMA xbar (dma_start_transpose), again no PE work.
  - pre-LN gamma/beta folded into Wq on host: Q = qn @ wq' + bq, bias
    added during the PSUM->SBUF evacuation (per-partition AP scalar).
  - Attention processes HEAD PAIRS: even head S matmuls use PE rows 0-63,
    odd head rows 64-127 (tile_position auto-derived from base_partition),
    so the two streams execute concurrently and the full array stays busy
    (keeps the HAM activity monitor at K=8/8).  S psum is one [P,2,QH]
    tile (2 banks); exp consumes both heads in one ACT pass per q-half.
  - PV with ones-augmented V (sumexp rides along as psum row 64), psum as
    a single [E,L] tile per head (2 banks, ring of 2).  PV of the previous
    pair is interleaved chunk-wise into the current pair's S block.
  - Sumexp epilogue: psum row 64 -> DMA to lane 0 -> approx-fast DVE
    reciprocal -> gpsimd partition_broadcast -> one DVE multiply into OT.
    No PE broadcast matmuls, no extra psum bank.
  - Output projection flipped token-major (stationary O^T chunks, moving
    Wo rows): tile t's j=0..6 matmuls run before tile t-1's j=7, and the
    LN chain (residual-add + row-sum fused in one DVE STT) trails by
    another tile.
"""

import numpy as np
import ml_dtypes

import concourse.bass as bass
import concourse.mybir as mybir
import concourse.tile as tile
from concourse import bacc
from concourse.dve_ops import RECIP_APPROX_FAST_CONSTS, RECIPROCAL_APPROX_FAST

P = 128
L = 1024          # tokens per batch element
D = 1024          # model dim
H = 16            # heads
HD = 64           # head dim
E = HD + 1        # head dim + sumexp column
NC = D // P       # 8 feature chunks
NT = L // P       # 8 token chunks
NQ = 2            # 512-wide halves of the moving/free dimension
QH = 512
EPS = 1e-6

FP32 = mybir.dt.float32
BF16 = mybir.dt.bfloat16
FP8 = mybir.dt.float8e4
FP32R = mybir.dt.float32r
OP = mybir.AluOpType
AF = mybir.ActivationFunctionType


def _emit_ln_stats(nc, pool, x, scratch, eps_t):
    """Return (rstd, neg_mu_rstd) per-partition [P,1] APs for LN of x."""
    st = pool.tile([P, 8], FP32, tag="lnst", bufs=4, name="lnst")
    nc.scalar.activation(scratch, x, AF.Copy, accum_out=st[:, 0:1])
    nc.scalar.activation(scratch, x, AF.Square, accum_out=st[:, 1:2])
    nc.vector.tensor_scalar_mul(st[:, 2:3], st[:, 0:1], 1.0 / D)     # mu
    nc.vector.tensor_tensor(st[:, 3:4], st[:, 2:3], st[:, 2:3], OP.mult)
    nc.vector.tensor_scalar_mul(st[:, 4:5], st[:, 1:2], 1.0 / D)     # E[x^2]
    nc.vector.tensor_tensor(st[:, 4:5], st[:, 4:5], st[:, 3:4], OP.subtract)
    nc.scalar.activation(st[:, 5:6], st[:, 4:5], AF.Sqrt, bias=eps_t)
    nc.vector.reciprocal(st[:, 6:7], st[:, 5:6])                     # rstd
    nc.vector.tensor_tensor(st[:, 7:8], st[:, 2:3], st[:, 6:7], OP.mult)
    nc.vector.tensor_scalar_mul(st[:, 7:8], st[:, 7:8], -1.0)        # -mu*rstd
    return st[:, 6:7], st[:, 7:8]


def build_bass():
    nc = bacc.Bacc("TRN2", target_bir_lowering=False, debug=False)

    qb_d = nc.dram_tensor("qb", [L, D], BF16, kind="ExternalInput")
    kt_d = nc.dram_tensor("kt", [D, L], BF16, kind="ExternalInput")
    vt_d = nc.dram_tensor("vt", [D, L], BF16, kind="ExternalInput")
    wq_d = nc.dram_tensor("wq", [D, D], BF16, kind="ExternalInput")
    wk_d = nc.dram_tensor("wk", [D, D], BF16, kind="ExternalInput")
    wv_d = nc.dram_tensor("wv", [D, D], BF16, kind="ExternalInput")
    wo_d = nc.dram_tensor("wo", [D, D], BF16, kind="ExternalInput")
    bq_d = nc.dram_tensor("bq", [P, NC], FP32, kind="ExternalInput")
    gb_d = nc.dram_tensor("gb", [P, D], FP32, kind="ExternalInput")
    bb_d = nc.dram_tensor("bb", [P, D], FP32, kind="ExternalInput")
    ep_d = nc.dram_tensor("epsc", [P, 1], FP32, kind="ExternalInput")
    vo_d = nc.dram_tensor("vone", [P, H * E], FP8, kind="ExternalInput")
    out_d = nc.dram_tensor("out", [L, D], FP32, kind="ExternalOutput")

    with tile.TileContext(nc) as tc:
        with tc.tile_pool(name="persist", bufs=1) as pp:
            eps_t = pp.tile([P, 1], FP32, name="eps_t")
            bq_t = pp.tile([P, NC], FP32, name="bq_t")
            KT = pp.tile([P, NC, L], BF16, name="KT")
            QT0 = pp.tile([P, 1, L], BF16, name="QT0")
            QTr = pp.tile([P, NC - 1, L], BF16, name="QTr")
            Vaug = pp.tile([P, NT, H * E], FP8, name="Vaug")
            OT = [pp.tile([P, L], BF16, name=f"ot{j}") for j in range(H // 2)]
            qb = [pp.tile([P, D], BF16, name=f"qb{t}") for t in range(NT)]

            nc.gpsimd.dma_start(eps_t, ep_d[:])
            nc.gpsimd.dma_start(bq_t, bq_d[:])

            # ---------------- QKV phase (V -> K -> Q) ----------------
            with (
                tc.tile_pool(name="qkv", bufs=1) as qp,
                tc.tile_pool(name="psA", bufs=1, space="PSUM") as psA,
            ):
                def load_w(dram, nm, eng):
                    tiles = []
                    for i in range(NC):
                        wt = qp.tile([P, D], BF16, tag="w", bufs=2 * NC,
                                     name=f"w{nm}{i}")
                        eng.dma_start(wt, dram[i * P:(i + 1) * P, :])
                        tiles.append(wt)
                    return tiles

                def xT_tile():
                    return qp.tile([P, NC, L], BF16, tag="xT", bufs=2,
                                   name="xT")

                # vT first (V-proj is the first PE phase)
                vT = xT_tile()
                for c in range(NC):
                    nc.sync.dma_start(vT[:, c, :], vt_d[c * P:(c + 1) * P, :])
                wv_t = load_w(wv_d, "v", nc.scalar)
                vo1 = qp.tile([P, H * E], FP8, name="vo1")
                nc.gpsimd.dma_start(vo1, vo_d[:])
                for t in range(NT):
                    nc.gpsimd.dma_start(Vaug[:, t, :], vo1)

                kT = xT_tile()
                for c in range(NC):
                    nc.sync.dma_start(kT[:, c, :], kt_d[c * P:(c + 1) * P, :])
                wk_t = load_w(wk_d, "k", nc.scalar)

                # q DMA + LN emitted early; ACT-side LN overlaps V/K-proj.
                # qb stays resident in bf16 and doubles as the residual.
                qn = []
                for t in range(NT):
                    nc.sync.dma_start(qb[t], qb_d[t * P:(t + 1) * P, :])
                    y = qp.tile([P, D], BF16, tag="qn", bufs=NT, name="qn")
                    rstd, nmr = _emit_ln_stats(nc, qp, qb[t], y, eps_t)
                    nc.scalar.activation(y, qb[t], AF.Identity, bias=nmr,
                                         scale=rstd)
                    qn.append(y)
                wq_t = load_w(wq_d, "q", nc.scalar)

                # ---- V-proj (token-major, into Vaug) ----
                for t in range(NT):
                    ps = psA.tile([P, L], FP32, tag="pj", bufs=2, name="ps_v")
                    for n in range(NQ):
                        for i in range(NC):
                            nc.tensor.matmul(
                                ps[:, n * QH:(n + 1) * QH],
                                vT[:, i, t * P:(t + 1) * P],
                                wv_t[i][:, n * QH:(n + 1) * QH],
                                start=(i == 0), stop=(i == NC - 1))
                    dst = Vaug[:, t, :].rearrange("p (h e) -> p h e", e=E)
                    nc.vector.tensor_copy(
                        dst[:, :, 0:HD],
                        ps.rearrange("p (h x) -> p h x", x=HD))

                def proj_feat(w_tiles, src, dst, bias_col=None):
                    dst_of = dst if callable(dst) else (
                        lambda m: dst[:, m, :])
                    for m in range(NC):
                        ps = psA.tile([P, L], FP32, tag="pj", bufs=2,
                                      name="ps_pj")
                        for n in range(NQ):
                            for i in range(NC):
                                nc.tensor.matmul(
                                    ps[:, n * QH:(n + 1) * QH],
                                    w_tiles[i][:, m * P:(m + 1) * P],
                                    src[:, i, n * QH:(n + 1) * QH],
                                    start=(i == 0), stop=(i == NC - 1))
                        if bias_col is None:
                            nc.vector.tensor_copy(dst_of(m), ps)
                        else:
                            nc.vector.tensor_scalar_add(
                                dst_of(m), ps, bias_col[:, m:m + 1])

                # ---- K-proj ----
                proj_feat(wk_t, kT, KT)

                # ---- qn -> qnT via DMA xbar transpose (no PE work) ----
                qnT = xT_tile()
                for t in range(NT):
                    for c in range(NC):
                        eng = nc.sync if (t * NC + c) % 2 == 0 else nc.scalar
                        eng.dma_start_transpose(
                            qnT[:, c, t * P:(t + 1) * P],
                            qn[t][:, c * P:(c + 1) * P])

                # ---- Q-proj (bias folded) ----
                proj_feat(
                    wq_t, qnT,
                    lambda m: QT0[:, 0, :] if m == 0 else QTr[:, m - 1, :],
                    bias_col=bq_t)

            # ---------------- out-phase inputs (emit DMAs early) ----------
            with tc.tile_pool(name="fin", bufs=1) as fp:
                gamma_bc = fp.tile([P, D], FP32, name="gamma_bc")
                beta_bc = fp.tile([P, D], FP32, name="beta_bc")
                nc.gpsimd.dma_start(gamma_bc, gb_d[:])
                nc.gpsimd.dma_start(beta_bc, bb_d[:])
                wo_t = []
                for j in range(NC):
                    wt = fp.tile([P, D], BF16, tag="wo", bufs=NC,
                                 name=f"wo{j}")
                    nc.scalar.dma_start(wt, wo_d[j * P:(j + 1) * P, :])
                    wo_t.append(wt)

                # ---------------- attention (paired heads) ----------------
                with (
                    tc.tile_pool(name="att", bufs=1) as ap,
                    tc.tile_pool(name="psS", bufs=1, space="PSUM") as psS,
                    tc.tile_pool(name="psO", bufs=1, space="PSUM") as psO,
                ):
                    rc = RECIP_APPROX_FAST_CONSTS

                    def pv_mms(po, hh, head, PT, i, n):
                        nc.tensor.matmul(
                            po[:, n * QH:(n + 1) * QH],
                            Vaug[:, i, head * E:(head + 1) * E],
                            PT[:, i, hh, n * QH:(n + 1) * QH],
                            start=(i == 0), stop=(i == NT - 1))

                    def epilogue_a(head, po):
                        """Drain one head's PV psum + kick off the
                        reciprocal chain (frees the psO slot fast).

                        The approx-fast DVE reciprocal only works at
                        partition 0, so the sumexp row goes psum(row 64)
                        -> sbuf(lane 64) -> DMA -> sbuf(lane 0)."""
                        rin = fp.tile([E, L], FP32, tag="rin", bufs=2,
                                      name="rin")
                        rec = fp.tile([1, L], FP32R, tag="rec", bufs=2,
                                      name="rec")
                        ou = fp.tile([HD, L], FP32, tag="ou", bufs=2,
                                     name="ou")
                        nc.vector.tensor_copy(rin[HD:E, :], po[HD:E, :])
                        nc.vector.tensor_copy(ou, po[0:HD, :])
                        nc.gpsimd.dma_start(rin[0:1, :], rin[HD:E, :])
                        nc.vector._custom_dve(
                            RECIPROCAL_APPROX_FAST, out=rec, in0=rin[0:1, :],
                            s0=rc["s0"], s1=rc["s1"], imm2=rc["imm2"])
                        return (head, ou, rec)

                    def epilogue_b(head, ou, rec):
                        """Broadcast 1/sumexp across partitions on the idle
                        GpSimd engine, then one DVE multiply into OT."""
                        c2, half = head // 2, head % 2
                        rbc = fp.tile([HD, L], FP32R, tag="rbc", bufs=2,
                                      name="rbc")
                        nc.gpsimd.partition_broadcast(rbc, rec[0:1, :],
                                                      channels=HD)
                        if half == 0:
                            nc.vector.tensor_tensor(
                                OT[c2][0:HD, :], ou, rbc, OP.mult)
                        else:
                            otmp = fp.tile([HD, L], BF16, tag="otmp",
                                           bufs=2, name="otmp")
                            nc.vector.tensor_tensor(otmp, ou, rbc, OP.mult)
                            nc.gpsimd.dma_start(OT[c2][HD:P, 0:QH],
                                                otmp[:, 0:QH])
                            nc.sync.dma_start(OT[c2][HD:P, QH:L],
                                              otmp[:, QH:L])

                    def s_block(c, PT, i, n):
                        """Paired S matmuls for (head 2c, 2c+1), q-half n,
                        key block i, + exp into PT."""
                        ss = psS.tile([P, 2, QH], FP32, tag="s", name="ps_s")
                        ks = slice(i * P, (i + 1) * P)
                        ns = slice(n * QH, (n + 1) * QH)
                        for hh in range(2):
                            hs = slice(hh * HD, hh * HD + HD)
                            qsrc = (QT0[hs, 0, ns] if c == 0 else
                                    QTr[hs, c - 1, ns])
                            nc.tensor.matmul(ss[:, hh, :], KT[hs, c, ks],
                                             qsrc, start=True, stop=True)
                        nc.scalar.activation(PT[:, i, :, ns], ss, AF.Exp)

                    prev = None   # (c, PT) pending PV
                    epis = []     # pending epilogue_b args
                    for c in range(H // 2):
                        PT = ap.tile([P, NC, 2, L], FP8, tag="pt", bufs=2,
                                     name="pt")
                        po_e = po_o = None
                        if prev is not None:
                            pc, pPT = prev
                            po_e = psO.tile([E, L], FP32, tag="o", bufs=2,
                                            name="ps_oe")
                            po_o = psO.tile([E, L], FP32, tag="o", bufs=2,
                                            name="ps_oo")
                        step = 0
                        for i in range(NT):
                            for n in range(NQ):
                                # PV of the previous pair goes first: the
                                # PE queue is in-order, so arriving at the
                                # psS-gated S matmuls later absorbs the exp
                                # semaphore latency.
                                if prev is not None:
                                    pv_mms(po_e, 0, 2 * pc, pPT, i, n)
                                    pv_mms(po_o, 1, 2 * pc + 1, pPT, i, n)
                                if step == 5 and epis:
                                    epilogue_b(*epis.pop(0))
                                if step == 9 and epis:
                                    epilogue_b(*epis.pop(0))
                                s_block(c, PT, i, n)
                                step += 1
                        if prev is not None:
                            epis.append(epilogue_a(2 * pc, po_e))
                            epis.append(epilogue_a(2 * pc + 1, po_o))
                        prev = (c, PT)

                    # drain: PV of the last pair; its epilogue_b is deferred
                    # into the out-projection warmup so the PE keeps busy
                    # while the reciprocal chains run.
                    while epis:
                        epilogue_b(*epis.pop(0))
                    pc, pPT = prev
                    po_e = psO.tile([E, L], FP32, tag="o", bufs=2,
                                    name="ps_oe")
                    po_o = psO.tile([E, L], FP32, tag="o", bufs=2,
                                    name="ps_oo")
                    for i in range(NT):
                        for n in range(NQ):
                            pv_mms(po_e, 0, 2 * pc, pPT, i, n)
                            pv_mms(po_o, 1, 2 * pc + 1, pPT, i, n)
                    last_epis = [epilogue_a(2 * pc, po_e),
                                 epilogue_a(2 * pc + 1, po_o)]

                # ------------- output projection + residual + LN ---------
                # Software-pipelined two deep: tile t's j=0..6 matmuls run
                # before tile t-1's j=7 (so the last head pair's OT has
                # time to land), and the LN chain trails by another tile.
                with tc.tile_pool(name="psW", bufs=3, space="PSUM") as psW:
                    def emit_j7_u(t, ps):
                        for n in range(NQ):
                            nc.tensor.matmul(
                                ps[:, n * QH:(n + 1) * QH],
                                OT[NC - 1][:, t * P:(t + 1) * P],
                                wo_t[NC - 1][:, n * QH:(n + 1) * QH],
                                start=False, stop=True)
                        u = fp.tile([P, D], FP32, tag="u", bufs=4, name="u")
                        st = fp.tile([P, 8], FP32, tag="lnst", bufs=3,
                                     name="lnst")
                        nc.vector.scalar_tensor_tensor(
                            u, ps, 0.0, qb[t], OP.add, OP.add,
                            accum_out=st[:, 0:1])
                        return (t, u, st)

                    def emit_ln_out(t, u, st):
                        """Mostly-DVE LN: one cross-engine hop (the tiny
                        sqrt), apply via two STT ops:
                        z = ((u - mu) * gamma) * rstd + beta."""
                        y = fp.tile([P, D], FP32, tag="y", bufs=3, name="y")
                        nc.scalar.activation(y, u, AF.Square,
                                             accum_out=st[:, 1:2])
                        nc.vector.tensor_scalar_mul(st[:, 2:3], st[:, 0:1],
                                                    1.0 / D)
                        nc.vector.tensor_tensor(st[:, 3:4], st[:, 2:3],
                                                st[:, 2:3], OP.mult)
                        nc.vector.tensor_scalar_mul(st[:, 4:5], st[:, 1:2],
                                                    1.0 / D)
                        nc.vector.tensor_tensor(st[:, 4:5], st[:, 4:5],
                                                st[:, 3:4], OP.subtract)
                        nc.scalar.activation(st[:, 5:6], st[:, 4:5],
                                             AF.Sqrt, bias=eps_t)
                        nc.vector.reciprocal(st[:, 6:7], st[:, 5:6])
                        nc.vector.scalar_tensor_tensor(
                            y, u, st[:, 2:3], gamma_bc,
                            OP.subtract, OP.mult)
                        z = fp.tile([P, D], FP32, tag="z", bufs=3, name="z")
                        nc.vector.scalar_tensor_tensor(
                            z, y, st[:, 6:7], beta_bc, OP.mult, OP.add)
                        ts_ = slice(t * P, (t + 1) * P)
                        nc.sync.dma_start(out_d[ts_, 0:QH], z[:, 0:QH])
                        nc.scalar.dma_start(out_d[ts_, QH:D], z[:, QH:D])

                    open_ps = {}
                    lnq = []
                    for t in range(NT):
                        ps = psW.tile([P, D], FP32, tag="w", name="ps_w")
                        for n in range(NQ):
                            for j in range(NC - 1):
                                nc.tensor.matmul(
                                    ps[:, n * QH:(n + 1) * QH],
                                    OT[j][:, t * P:(t + 1) * P],
                                    wo_t[j][:, n * QH:(n + 1) * QH],
                                    start=(j == 0), stop=False)
                        open_ps[t] = ps
                        if t == 1:
                            epilogue_b(*last_epis.pop(0))
                        if t == 2:
                            epilogue_b(*last_epis.pop(0))
                        if t - 1 in open_ps:
                            lnq.append(emit_j7_u(t - 1, open_ps.pop(t - 1)))
                        if len(lnq) > 0:
                            emit_ln_out(*lnq.pop(0))
                    lnq.append(emit_j7_u(NT - 1, open_ps.pop(NT - 1)))
                    for args in lnq:
                        emit_ln_out(*args)

    nc.compile()
    return nc


_CACHE = {}


def _get_nc():
    if "nc" not in _CACHE:
        _CACHE["nc"] = build_bass()
    return _CACHE["nc"]


def make_in_maps(q, k, v, Wq, Wk, Wv, Wo, gamma, beta):
    qb = np.asarray(q, np.float32).astype(ml_dtypes.bfloat16)
    kb = np.asarray(k, np.float32).astype(ml_dtypes.bfloat16)
    vb = np.asarray(v, np.float32).astype(ml_dtypes.bfloat16)
    gamma = np.asarray(gamma, np.float32)
    beta = np.asarray(beta, np.float32)
    Wq = np.asarray(Wq, np.float32)
    # fold pre-LN gamma/beta and the 1/sqrt(dk)=0.125 scale into Wq
    wq = (0.125 * gamma[:, None] * Wq).astype(ml_dtypes.bfloat16)
    bq = (0.125 * (beta @ Wq)).astype(np.float32)           # [D]
    bq_t = np.ascontiguousarray(bq.reshape(NC, P).T)        # [P, NC]
    wk = np.asarray(Wk, np.float32).astype(ml_dtypes.bfloat16)
    wv = np.asarray(Wv, np.float32).astype(ml_dtypes.bfloat16)
    wo = np.asarray(Wo, np.float32).astype(ml_dtypes.bfloat16)
    gb = np.ascontiguousarray(np.tile(gamma[None, :], (P, 1)))
    bb = np.ascontiguousarray(np.tile(beta[None, :], (P, 1)))
    epsc = np.full((P, 1), EPS, np.float32)
    vone = np.ones((P, H * E), ml_dtypes.float8_e4m3fn)
    B = q.shape[0]
    return [
        {
            "qb": np.ascontiguousarray(qb[b]),
            "kt": np.ascontiguousarray(kb[b].T),
            "vt": np.ascontiguousarray(vb[b].T),
            "wq": wq, "wk": wk, "wv": wv, "wo": wo, "bq": bq_t,
            "gb": gb, "bb": bb, "epsc": epsc, "vone": vone,
        }
        for b in range(B)
    ]


def kernel(q, k, v, Wq, Wk, Wv, Wo, gamma, beta, trace=False):
    from concourse.bass_utils import run_bass_kernel_spmd

    nc = _get_nc()
    in_maps = make_in_maps(q, k, v, Wq, Wk, Wv, Wo, gamma, beta)
    res = run_bass_kernel_spmd(nc, in_maps, core_ids=list(range(len(in_maps))),
                               trace=trace)
    out = np.stack([r["out"] for r in res.results], axis=0)
    if trace:
        return out, res
    return out


# revision 16
# speedup vs baseline: 1.0785x; 1.0785x over previous
"""Trainium2 Bass kernel: pre-LN multi-head attention block (B=8, L=1024,
D=1024, H=16, dk=dv=64), data-parallel over batch across 8 NeuronCores.

Design (v2 lineage: no PE transposes, paired S matmuls):
  - q/k/v pre-cast to bf16 on host; k and v are transposed on the HOST, so
    the kernel DMAs kT/vT directly and the PE never runs a transpose for
    them. q stays token-major (residual + LN stats); qn is LN'd on ACT and
    transposed by the DMA xbar (dma_start_transpose), again no PE work.
  - pre-LN gamma/beta folded into Wq on host: Q = qn @ wq' + bq, bias
    added during the PSUM->SBUF evacuation (per-partition AP scalar).
  - Attention processes HEAD PAIRS: even head S matmuls use PE rows 0-63,
    odd head rows 64-127 (tile_position auto-derived from base_partition),
    so the two streams execute concurrently and the full array stays busy
    (keeps the HAM activity monitor at K=8/8).  S psum is one [P,2,QH]
    tile (2 banks); exp consumes both heads in one ACT pass per q-half.
  - PV with ones-augmented V (sumexp rides along as psum row 64), psum as
    a single [E,L] tile per head (2 banks, ring of 2).  PV of the previous
    pair is interleaved chunk-wise into the current pair's S block.
  - Sumexp epilogue: psum row 64 -> DMA to lane 0 -> approx-fast DVE
    reciprocal -> gpsimd partition_broadcast -> one DVE multiply into OT.
    No PE broadcast matmuls, no extra psum bank.
  - Output projection flipped token-major (stationary O^T chunks, moving
    Wo rows): tile t's j=0..6 matmuls run before tile t-1's j=7, and the
    LN chain (residual-add + row-sum fused in one DVE STT) trails by
    another tile.
"""

import numpy as np
import ml_dtypes

import concourse.bass as bass
import concourse.mybir as mybir
import concourse.tile as tile
from concourse import bacc
from concourse.dve_ops import RECIP_APPROX_FAST_CONSTS, RECIPROCAL_APPROX_FAST

P = 128
L = 1024          # tokens per batch element
D = 1024          # model dim
H = 16            # heads
HD = 64           # head dim
E = HD + 1        # head dim + sumexp column
NC = D // P       # 8 feature chunks
NT = L // P       # 8 token chunks
NQ = 2            # 512-wide halves of the moving/free dimension
QH = 512
EPS = 1e-6
DEBUG = False

FP32 = mybir.dt.float32
BF16 = mybir.dt.bfloat16
FP8 = mybir.dt.float8e4
FP32R = mybir.dt.float32r
OP = mybir.AluOpType
AF = mybir.ActivationFunctionType


def _emit_ln_stats(nc, pool, x, scratch, eps_t):
    """Return (rstd, neg_mu_rstd) per-partition [P,1] APs for LN of x."""
    st = pool.tile([P, 8], FP32, tag="lnst", bufs=4, name="lnst")
    nc.scalar.activation(scratch, x, AF.Copy, accum_out=st[:, 0:1])
    nc.scalar.activation(scratch, x, AF.Square, accum_out=st[:, 1:2])
    nc.vector.tensor_scalar_mul(st[:, 2:3], st[:, 0:1], 1.0 / D)     # mu
    nc.vector.tensor_tensor(st[:, 3:4], st[:, 2:3], st[:, 2:3], OP.mult)
    nc.vector.tensor_scalar_mul(st[:, 4:5], st[:, 1:2], 1.0 / D)     # E[x^2]
    nc.vector.tensor_tensor(st[:, 4:5], st[:, 4:5], st[:, 3:4], OP.subtract)
    nc.scalar.activation(st[:, 5:6], st[:, 4:5], AF.Sqrt, bias=eps_t)
    nc.vector.reciprocal(st[:, 6:7], st[:, 5:6])                     # rstd
    nc.vector.tensor_tensor(st[:, 7:8], st[:, 2:3], st[:, 6:7], OP.mult)
    nc.vector.tensor_scalar_mul(st[:, 7:8], st[:, 7:8], -1.0)        # -mu*rstd
    return st[:, 6:7], st[:, 7:8]


def build_bass():
    nc = bacc.Bacc("TRN2", target_bir_lowering=False, debug=False)

    qb_d = nc.dram_tensor("qb", [L, D], BF16, kind="ExternalInput")
    kt_d = nc.dram_tensor("kt", [D, L], BF16, kind="ExternalInput")
    vt_d = nc.dram_tensor("vt", [D, L], BF16, kind="ExternalInput")
    wq_d = nc.dram_tensor("wq", [D, D], BF16, kind="ExternalInput")
    wk_d = nc.dram_tensor("wk", [D, D], BF16, kind="ExternalInput")
    wv_d = nc.dram_tensor("wv", [D, D], BF16, kind="ExternalInput")
    wo_d = nc.dram_tensor("wo", [D, D], BF16, kind="ExternalInput")
    bq_d = nc.dram_tensor("bq", [P, NC], FP32, kind="ExternalInput")
    gb_d = nc.dram_tensor("gb", [P, D], FP32, kind="ExternalInput")
    bb_d = nc.dram_tensor("bb", [P, D], FP32, kind="ExternalInput")
    ep_d = nc.dram_tensor("epsc", [P, 1], FP32, kind="ExternalInput")
    n2_d = nc.dram_tensor("neg2", [P, 1], FP32, kind="ExternalInput")
    vo_d = nc.dram_tensor("vone", [P, H * E], BF16, kind="ExternalInput")
    out_d = nc.dram_tensor("out", [L, D], FP32, kind="ExternalOutput")
    dbg = {}
    if DEBUG:
        dbg["qnT"] = nc.dram_tensor("dqnT", [P, NC * L], BF16,
                                    kind="ExternalOutput")
        dbg["KT"] = nc.dram_tensor("dKT", [P, NC * L], BF16,
                                   kind="ExternalOutput")
        dbg["QT"] = nc.dram_tensor("dQT", [P, NC * L], BF16,
                                   kind="ExternalOutput")
        dbg["Va"] = nc.dram_tensor("dVa", [P, NT * H * E], FP8,
                                   kind="ExternalOutput")
        dbg["OT0"] = nc.dram_tensor("dOT0", [P, L], BF16,
                                    kind="ExternalOutput")
        dbg["OT7"] = nc.dram_tensor("dOT7", [P, L], BF16,
                                    kind="ExternalOutput")

    with tile.TileContext(nc) as tc:
        with tc.tile_pool(name="persist", bufs=1) as pp:
            eps_t = pp.tile([P, 1], FP32, name="eps_t")
            neg2_t = pp.tile([P, 1], FP32, name="neg2_t")
            bq_t = pp.tile([P, NC], FP32, name="bq_t")
            KT = pp.tile([P, NC, L], BF16, name="KT")
            QT0 = pp.tile([P, 1, L], BF16, name="QT0")
            QTr = pp.tile([P, NC - 1, L], BF16, name="QTr")
            Vaug = pp.tile([P, NT, H * E], BF16, name="Vaug")
            OT = [pp.tile([P, L], BF16, name=f"ot{j}") for j in range(H // 2)]
            qb = [pp.tile([P, D], BF16, name=f"qb{t}") for t in range(NT)]

            nc.gpsimd.dma_start(eps_t, ep_d[:])
            nc.gpsimd.dma_start(neg2_t, n2_d[:])
            nc.gpsimd.dma_start(bq_t, bq_d[:])

            # ---------------- QKV phase (V -> K -> Q) ----------------
            with (
                tc.tile_pool(name="qkv", bufs=1) as qp,
                tc.tile_pool(name="psA", bufs=1, space="PSUM") as psA,
            ):
                def load_w(dram, nm, eng):
                    tiles = []
                    for i in range(NC):
                        wt = qp.tile([P, D], BF16, tag="w", bufs=2 * NC,
                                     name=f"w{nm}{i}")
                        eng.dma_start(wt, dram[i * P:(i + 1) * P, :])
                        tiles.append(wt)
                    return tiles

                def xT_tile():
                    return qp.tile([P, NC, L], BF16, tag="xT", bufs=2,
                                   name="xT")

                # vT first (V-proj is the first PE phase)
                vT = xT_tile()
                for c in range(NC):
                    nc.sync.dma_start(vT[:, c, :], vt_d[c * P:(c + 1) * P, :])
                wv_t = load_w(wv_d, "v", nc.scalar)
                vo1 = qp.tile([P, H * E], BF16, name="vo1")
                nc.gpsimd.dma_start(vo1, vo_d[:])
                for t in range(NT):
                    nc.gpsimd.dma_start(Vaug[:, t, :], vo1)

                kT = xT_tile()
                for c in range(NC):
                    nc.sync.dma_start(kT[:, c, :], kt_d[c * P:(c + 1) * P, :])
                wk_t = load_w(wk_d, "k", nc.scalar)

                # q DMA + LN emitted early; ACT-side LN overlaps V/K-proj.
                # qb stays resident in bf16 and doubles as the residual.
                qn = []
                for t in range(NT):
                    nc.sync.dma_start(qb[t], qb_d[t * P:(t + 1) * P, :])
                    y = qp.tile([P, D], BF16, tag="qn", bufs=NT, name="qn")
                    rstd, nmr = _emit_ln_stats(nc, qp, qb[t], y, eps_t)
                    nc.scalar.activation(y, qb[t], AF.Identity, bias=nmr,
                                         scale=rstd)
                    qn.append(y)
                wq_t = load_w(wq_d, "q", nc.scalar)

                # ---- V-proj (token-major, into Vaug) ----
                for t in range(NT):
                    ps = psA.tile([P, L], FP32, tag="pj", bufs=2, name="ps_v")
                    for n in range(NQ):
                        for i in range(NC):
                            nc.tensor.matmul(
                                ps[:, n * QH:(n + 1) * QH],
                                vT[:, i, t * P:(t + 1) * P],
                                wv_t[i][:, n * QH:(n + 1) * QH],
                                start=(i == 0), stop=(i == NC - 1))
                    dst = Vaug[:, t, :].rearrange("p (h e) -> p h e", e=E)
                    nc.vector.tensor_copy(
                        dst[:, :, 0:HD],
                        ps.rearrange("p (h x) -> p h x", x=HD))

                def proj_feat(w_tiles, src, dst, bias_col=None):
                    dst_of = dst if callable(dst) else (
                        lambda m: dst[:, m, :])
                    for m in range(NC):
                        ps = psA.tile([P, L], FP32, tag="pj", bufs=2,
                                      name="ps_pj")
                        for n in range(NQ):
                            for i in range(NC):
                                nc.tensor.matmul(
                                    ps[:, n * QH:(n + 1) * QH],
                                    w_tiles[i][:, m * P:(m + 1) * P],
                                    src[:, i, n * QH:(n + 1) * QH],
                                    start=(i == 0), stop=(i == NC - 1))
                        if bias_col is None:
                            nc.vector.tensor_copy(dst_of(m), ps)
                        else:
                            nc.vector.tensor_scalar_add(
                                dst_of(m), ps, bias_col[:, m:m + 1])

                # ---- K-proj ----
                proj_feat(wk_t, kT, KT)

                # ---- qn -> qnT via DMA xbar transpose (no PE work) ----
                # xbar transpose needs a per-partition-contiguous dst, so
                # stage [P,NC,P] then scatter into qnT with a plain DMA.
                qnT = xT_tile()
                for t in range(NT):
                    stage = qp.tile([P, NC, P], BF16, tag="tstg", bufs=2,
                                    name="tstg")
                    eng = nc.sync if t % 2 == 0 else nc.scalar
                    eng.dma_start_transpose(stage, qn[t])
                    eng.dma_start(qnT[:, :, t * P:(t + 1) * P], stage)

                # ---- Q-proj (bias folded) ----
                proj_feat(
                    wq_t, qnT,
                    lambda m: QT0[:, 0, :] if m == 0 else QTr[:, m - 1, :],
                    bias_col=bq_t)
                if DEBUG:
                    nc.gpsimd.dma_start(
                        dbg["qnT"][:], qnT.rearrange("p c l -> p (c l)"))
                    nc.gpsimd.dma_start(
                        dbg["KT"][:], KT.rearrange("p c l -> p (c l)"))
                    nc.gpsimd.dma_start(dbg["QT"][:, 0:L], QT0[:, 0, :])
                    nc.gpsimd.dma_start(
                        dbg["QT"][:, L:], QTr.rearrange("p c l -> p (c l)"))
                    nc.gpsimd.dma_start(
                        dbg["Va"][:], Vaug.rearrange("p t e -> p (t e)"))

            # ---------------- out-phase inputs (emit DMAs early) ----------
            with tc.tile_pool(name="fin", bufs=1) as fp:
                gamma_bc = fp.tile([P, D], FP32, name="gamma_bc")
                beta_bc = fp.tile([P, D], FP32, name="beta_bc")
                nc.gpsimd.dma_start(gamma_bc, gb_d[:])
                nc.gpsimd.dma_start(beta_bc, bb_d[:])
                wo_t = []
                for j in range(NC):
                    wt = fp.tile([P, D], BF16, tag="wo", bufs=NC,
                                 name=f"wo{j}")
                    nc.scalar.dma_start(wt, wo_d[j * P:(j + 1) * P, :])
                    wo_t.append(wt)

                # ---------------- attention (paired heads) ----------------
                with (
                    tc.tile_pool(name="att", bufs=1) as ap,
                    tc.tile_pool(name="psS", bufs=1, space="PSUM") as psS,
                    tc.tile_pool(name="psO", bufs=1, space="PSUM") as psO,
                ):
                    rc = RECIP_APPROX_FAST_CONSTS

                    def pv_mms(po, hh, head, PT, i, n):
                        nc.tensor.matmul(
                            po[:, n * QH:(n + 1) * QH],
                            Vaug[:, i, head * E:(head + 1) * E],
                            PT[:, i, hh, n * QH:(n + 1) * QH],
                            start=(i == 0), stop=(i == NT - 1))

                    def epilogue_a(head, po):
                        """Drain one head's PV psum + kick off the
                        reciprocal chain (frees the psO slot fast).

                        The approx-fast DVE reciprocal only works at
                        partition 0, so the sumexp row goes psum(row 64)
                        -> sbuf(lane 64) -> DMA -> sbuf(lane 0)."""
                        rin = fp.tile([E, L], FP32, tag="rin", bufs=2,
                                      name="rin")
                        rec = fp.tile([1, L], FP32R, tag="rec", bufs=2,
                                      name="rec")
                        ou = fp.tile([HD, L], FP32, tag="ou", bufs=2,
                                     name="ou")
                        nc.vector.tensor_copy(rin[HD:E, :], po[HD:E, :])
                        nc.vector.tensor_copy(ou, po[0:HD, :])
                        nc.gpsimd.dma_start(rin[0:1, :], rin[HD:E, :])
                        nc.vector._custom_dve(
                            RECIPROCAL_APPROX_FAST, out=rec, in0=rin[0:1, :],
                            s0=rc["s0"], s1=rc["s1"], imm2=rc["imm2"])
                        return (head, ou, rec)

                    def epilogue_b(head, ou, rec):
                        """Broadcast 1/sumexp across partitions on the idle
                        GpSimd engine, then one DVE multiply into OT."""
                        c2, half = head // 2, head % 2
                        rbc = fp.tile([HD, L], FP32R, tag="rbc", bufs=2,
                                      name="rbc")
                        nc.gpsimd.partition_broadcast(rbc, rec[0:1, :],
                                                      channels=HD)
                        if half == 0:
                            nc.vector.tensor_tensor(
                                OT[c2][0:HD, :], ou, rbc, OP.mult)
                        else:
                            otmp = fp.tile([HD, L], BF16, tag="otmp",
                                           bufs=2, name="otmp")
                            nc.vector.tensor_tensor(otmp, ou, rbc, OP.mult)
                            nc.gpsimd.dma_start(OT[c2][HD:P, 0:QH],
                                                otmp[:, 0:QH])
                            nc.sync.dma_start(OT[c2][HD:P, QH:L],
                                              otmp[:, QH:L])

                    def s_block(c, PT, i, n):
                        """Paired S matmuls for (head 2c, 2c+1), q-half n,
                        key block i, + exp into PT."""
                        ss = psS.tile([P, 2, QH], FP32, tag="s", name="ps_s")
                        ks = slice(i * P, (i + 1) * P)
                        ns = slice(n * QH, (n + 1) * QH)
                        for hh in range(2):
                            hs = slice(hh * HD, hh * HD + HD)
                            qsrc = (QT0[hs, 0, ns] if c == 0 else
                                    QTr[hs, c - 1, ns])
                            nc.tensor.matmul(ss[:, hh, :], KT[hs, c, ks],
                                             qsrc, start=True, stop=True)
                        # exp(S-3): shift keeps exp within fp8e4m3 range
                        # (overflow needs S>9.1; observed max 7.6+noise);
                        # softmax is shift-invariant and sumexp sums the
                        # shifted values.
                        nc.scalar.activation(PT[:, i, :, ns], ss, AF.Exp,
                                             bias=neg2_t)

                    prev = None   # (c, PT) pending PV
                    epis = []     # pending epilogue_b args
                    for c in range(H // 2):
                        PT = ap.tile([P, NC, 2, L], FP8, tag="pt", bufs=2,
                                     name="pt")
                        po_e = po_o = None
                        if prev is not None:
                            pc, pPT = prev
                            po_e = psO.tile([E, L], FP32, tag="o", bufs=3,
                                            name="ps_oe")
                            po_o = psO.tile([E, L], FP32, tag="o", bufs=3,
                                            name="ps_oo")
                        step = 0
                        for i in range(NT):
                            for n in range(NQ):
                                # PV of the previous pair goes first: the
                                # PE queue is in-order, so arriving at the
                                # psS-gated S matmuls later absorbs the exp
                                # semaphore latency.
                                if prev is not None:
                                    pv_mms(po_e, 0, 2 * pc, pPT, i, n)
                                    pv_mms(po_o, 1, 2 * pc + 1, pPT, i, n)
                                if step == 5 and epis:
                                    epilogue_b(*epis.pop(0))
                                if step == 9 and epis:
                                    epilogue_b(*epis.pop(0))
                                s_block(c, PT, i, n)
                                step += 1
                        if prev is not None:
                            epis.append(epilogue_a(2 * pc, po_e))
                            epis.append(epilogue_a(2 * pc + 1, po_o))
                        prev = (c, PT)

                    # drain: PV of the last pair; its epilogue_b is deferred
                    # into the out-projection warmup so the PE keeps busy
                    # while the reciprocal chains run.
                    while epis:
                        epilogue_b(*epis.pop(0))
                    pc, pPT = prev
                    po_e = psO.tile([E, L], FP32, tag="o", bufs=3,
                                    name="ps_oe")
                    po_o = psO.tile([E, L], FP32, tag="o", bufs=3,
                                    name="ps_oo")
                    for i in range(NT):
                        for n in range(NQ):
                            pv_mms(po_e, 0, 2 * pc, pPT, i, n)
                            pv_mms(po_o, 1, 2 * pc + 1, pPT, i, n)
                    last_epis = [epilogue_a(2 * pc, po_e),
                                 epilogue_a(2 * pc + 1, po_o)]

                if DEBUG:
                    nc.gpsimd.dma_start(dbg["OT0"][:], OT[0][:])
                    nc.gpsimd.dma_start(dbg["OT7"][:], OT[7][:])

                # ------------- output projection + residual + LN ---------
                # Software-pipelined two deep: tile t's j=0..6 matmuls run
                # before tile t-1's j=7 (so the last head pair's OT has
                # time to land), and the LN chain trails by another tile.
                with tc.tile_pool(name="psW", bufs=3, space="PSUM") as psW:
                    def emit_j7_u(t, ps):
                        for n in range(NQ):
                            nc.tensor.matmul(
                                ps[:, n * QH:(n + 1) * QH],
                                OT[NC - 1][:, t * P:(t + 1) * P],
                                wo_t[NC - 1][:, n * QH:(n + 1) * QH],
                                start=False, stop=True)
                        u = fp.tile([P, D], FP32, tag="u", bufs=3, name="u")
                        st = fp.tile([P, 8], FP32, tag="lnst", bufs=3,
                                     name="lnst")
                        nc.vector.scalar_tensor_tensor(
                            u, ps, 0.0, qb[t], OP.add, OP.add,
                            accum_out=st[:, 0:1])
                        return (t, u, st)

                    def emit_ln_out(t, u, st):
                        """Mostly-DVE LN: one cross-engine hop (the tiny
                        sqrt), apply via two STT ops:
                        z = ((u - mu) * gamma) * rstd + beta."""
                        y = fp.tile([P, D], FP32, tag="y", bufs=2, name="y")
                        nc.scalar.activation(y, u, AF.Square,
                                             accum_out=st[:, 1:2])
                        nc.vector.tensor_scalar_mul(st[:, 2:3], st[:, 0:1],
                                                    1.0 / D)
                        nc.vector.tensor_tensor(st[:, 3:4], st[:, 2:3],
                                                st[:, 2:3], OP.mult)
                        nc.vector.tensor_scalar_mul(st[:, 4:5], st[:, 1:2],
                                                    1.0 / D)
                        nc.vector.tensor_tensor(st[:, 4:5], st[:, 4:5],
                                                st[:, 3:4], OP.subtract)
                        nc.scalar.activation(st[:, 5:6], st[:, 4:5],
                                             AF.Sqrt, bias=eps_t)
                        nc.vector.reciprocal(st[:, 6:7], st[:, 5:6])
                        nc.vector.scalar_tensor_tensor(
                            y, u, st[:, 2:3], gamma_bc,
                            OP.subtract, OP.mult)
                        z = fp.tile([P, D], FP32, tag="z", bufs=3, name="z")
                        nc.vector.scalar_tensor_tensor(
                            z, y, st[:, 6:7], beta_bc, OP.mult, OP.add)
                        ts_ = slice(t * P, (t + 1) * P)
                        nc.sync.dma_start(out_d[ts_, 0:QH], z[:, 0:QH])
                        nc.scalar.dma_start(out_d[ts_, QH:D], z[:, QH:D])

                    open_ps = {}
                    lnq = []
                    for t in range(NT):
                        ps = psW.tile([P, D], FP32, tag="w", name="ps_w")
                        for n in range(NQ):
                            for j in range(NC - 1):
                                nc.tensor.matmul(
                                    ps[:, n * QH:(n + 1) * QH],
                                    OT[j][:, t * P:(t + 1) * P],
                                    wo_t[j][:, n * QH:(n + 1) * QH],
                                    start=(j == 0), stop=False)
                        open_ps[t] = ps
                        if t == 1:
                            epilogue_b(*last_epis.pop(0))
                        if t == 2:
                            epilogue_b(*last_epis.pop(0))
                        if t - 1 in open_ps:
                            lnq.append(emit_j7_u(t - 1, open_ps.pop(t - 1)))
                        if len(lnq) > 0:
                            emit_ln_out(*lnq.pop(0))
                    lnq.append(emit_j7_u(NT - 1, open_ps.pop(NT - 1)))
                    for args in lnq:
                        emit_ln_out(*args)

    nc.compile()
    return nc


_CACHE = {}


def _get_nc():
    if "nc" not in _CACHE:
        _CACHE["nc"] = build_bass()
    return _CACHE["nc"]


def make_in_maps(q, k, v, Wq, Wk, Wv, Wo, gamma, beta):
    qb = np.asarray(q, np.float32).astype(ml_dtypes.bfloat16)
    kb = np.asarray(k, np.float32).astype(ml_dtypes.bfloat16)
    vb = np.asarray(v, np.float32).astype(ml_dtypes.bfloat16)
    gamma = np.asarray(gamma, np.float32)
    beta = np.asarray(beta, np.float32)
    Wq = np.asarray(Wq, np.float32)
    # fold pre-LN gamma/beta and the 1/sqrt(dk)=0.125 scale into Wq
    wq = (0.125 * gamma[:, None] * Wq).astype(ml_dtypes.bfloat16)
    bq = (0.125 * (beta @ Wq)).astype(np.float32)           # [D]
    bq_t = np.ascontiguousarray(bq.reshape(NC, P).T)        # [P, NC]
    wk = np.asarray(Wk, np.float32).astype(ml_dtypes.bfloat16)
    wv = np.asarray(Wv, np.float32).astype(ml_dtypes.bfloat16)
    wo = np.asarray(Wo, np.float32).astype(ml_dtypes.bfloat16)
    gb = np.ascontiguousarray(np.tile(gamma[None, :], (P, 1)))
    bb = np.ascontiguousarray(np.tile(beta[None, :], (P, 1)))
    epsc = np.full((P, 1), EPS, np.float32)
    neg2 = np.full((P, 1), -3.0, np.float32)
    vone = np.ones((P, H * E), ml_dtypes.bfloat16)
    B = q.shape[0]
    return [
        {
            "qb": np.ascontiguousarray(qb[b]),
            "kt": np.ascontiguousarray(kb[b].T),
            "vt": np.ascontiguousarray(vb[b].T),
            "wq": wq, "wk": wk, "wv": wv, "wo": wo, "bq": bq_t,
            "gb": gb, "bb": bb, "epsc": epsc, "vone": vone, "neg2": neg2,
        }
        for b in range(B)
    ]


def kernel(q, k, v, Wq, Wk, Wv, Wo, gamma, beta, trace=False):
    from concourse.bass_utils import run_bass_kernel_spmd

    nc = _get_nc()
    in_maps = make_in_maps(q, k, v, Wq, Wk, Wv, Wo, gamma, beta)
    res = run_bass_kernel_spmd(nc, in_maps, core_ids=list(range(len(in_maps))),
                               trace=trace)
    out = np.stack([r["out"] for r in res.results], axis=0)
    if trace:
        return out, res
    return out


# revision 18
# speedup vs baseline: 1.3404x; 1.2429x over previous
"""Trainium2 Bass kernel: pre-LN multi-head attention block (B=8, L=1024,
D=1024, H=16, dk=dv=64), data-parallel over batch across 8 NeuronCores.

Design (v2 lineage: no PE transposes, paired S matmuls):
  - q/k/v pre-cast to bf16 on host; k and v are transposed on the HOST, so
    the kernel DMAs kT/vT directly and the PE never runs a transpose for
    them. q stays token-major (residual + LN stats); qn is LN'd on ACT and
    transposed by the DMA xbar (dma_start_transpose), again no PE work.
  - pre-LN gamma/beta folded into Wq on host: Q = qn @ wq' + bq, bias
    added during the PSUM->SBUF evacuation (per-partition AP scalar).
  - Attention processes HEAD PAIRS: even head S matmuls use PE rows 0-63,
    odd head rows 64-127 (tile_position auto-derived from base_partition),
    so the two streams execute concurrently and the full array stays busy
    (keeps the HAM activity monitor at K=8/8).  S psum is one [P,2,QH]
    tile (2 banks); exp consumes both heads in one ACT pass per q-half.
  - PV with ones-augmented V (sumexp rides along as psum row 64), psum as
    a single [E,L] tile per head (2 banks, ring of 2).  PV of the previous
    pair is interleaved chunk-wise into the current pair's S block.
  - Sumexp epilogue: psum row 64 -> DMA to lane 0 -> approx-fast DVE
    reciprocal -> gpsimd partition_broadcast -> one DVE multiply into OT.
    No PE broadcast matmuls, no extra psum bank.
  - Output projection flipped token-major (stationary O^T chunks, moving
    Wo rows): tile t's j=0..6 matmuls run before tile t-1's j=7, and the
    LN chain (residual-add + row-sum fused in one DVE STT) trails by
    another tile.
"""

import numpy as np
import ml_dtypes

import concourse.bass as bass
import concourse.mybir as mybir
import concourse.tile as tile
from concourse import bacc
from concourse.dve_ops import RECIP_APPROX_FAST_CONSTS, RECIPROCAL_APPROX_FAST

P = 128
L = 1024          # tokens per batch element
D = 1024          # model dim
H = 16            # heads
HD = 64           # head dim
E = HD + 1        # head dim + sumexp column
NC = D // P       # 8 feature chunks
NT = L // P       # 8 token chunks
NQ = 2            # 512-wide halves of the moving/free dimension
QH = 512
EPS = 1e-6
DEBUG = False

FP32 = mybir.dt.float32
BF16 = mybir.dt.bfloat16
FP8 = mybir.dt.float8e4
FP32R = mybir.dt.float32r
OP = mybir.AluOpType
AF = mybir.ActivationFunctionType


def _emit_ln_stats(nc, pool, x, scratch, eps_t):
    """Return (rstd, neg_mu_rstd) per-partition [P,1] APs for LN of x."""
    st = pool.tile([P, 8], FP32, tag="lnst", bufs=4, name="lnst")
    nc.scalar.activation(scratch, x, AF.Copy, accum_out=st[:, 0:1])
    nc.scalar.activation(scratch, x, AF.Square, accum_out=st[:, 1:2])
    nc.vector.tensor_scalar_mul(st[:, 2:3], st[:, 0:1], 1.0 / D)     # mu
    nc.vector.tensor_tensor(st[:, 3:4], st[:, 2:3], st[:, 2:3], OP.mult)
    nc.vector.tensor_scalar_mul(st[:, 4:5], st[:, 1:2], 1.0 / D)     # E[x^2]
    nc.vector.tensor_tensor(st[:, 4:5], st[:, 4:5], st[:, 3:4], OP.subtract)
    nc.scalar.activation(st[:, 5:6], st[:, 4:5], AF.Sqrt, bias=eps_t)
    nc.vector.reciprocal(st[:, 6:7], st[:, 5:6])                     # rstd
    nc.vector.tensor_tensor(st[:, 7:8], st[:, 2:3], st[:, 6:7], OP.mult)
    nc.vector.tensor_scalar_mul(st[:, 7:8], st[:, 7:8], -1.0)        # -mu*rstd
    return st[:, 6:7], st[:, 7:8]


def build_bass():
    nc = bacc.Bacc("TRN2", target_bir_lowering=False, debug=False)

    qb_d = nc.dram_tensor("qb", [L, D], BF16, kind="ExternalInput")
    kt_d = nc.dram_tensor("kt", [D, L], BF16, kind="ExternalInput")
    vt_d = nc.dram_tensor("vt", [D, L], BF16, kind="ExternalInput")
    wq_d = nc.dram_tensor("wq", [D, D], BF16, kind="ExternalInput")
    wk_d = nc.dram_tensor("wk", [D, D], BF16, kind="ExternalInput")
    wv_d = nc.dram_tensor("wv", [D, D], BF16, kind="ExternalInput")
    wo_d = nc.dram_tensor("wo", [D, D], BF16, kind="ExternalInput")
    bq_d = nc.dram_tensor("bq", [P, NC], FP32, kind="ExternalInput")
    gb_d = nc.dram_tensor("gb", [P, D], FP32, kind="ExternalInput")
    bb_d = nc.dram_tensor("bb", [P, D], FP32, kind="ExternalInput")
    ep_d = nc.dram_tensor("epsc", [P, 1], FP32, kind="ExternalInput")
    n2_d = nc.dram_tensor("neg2", [P, 1], FP32, kind="ExternalInput")
    vo_d = nc.dram_tensor("vone", [P, H * E], FP8, kind="ExternalInput")
    out_d = nc.dram_tensor("out", [L, D], FP32, kind="ExternalOutput")
    dbg = {}
    if DEBUG:
        dbg["qnT"] = nc.dram_tensor("dqnT", [P, NC * L], BF16,
                                    kind="ExternalOutput")
        dbg["KT"] = nc.dram_tensor("dKT", [P, NC * L], BF16,
                                   kind="ExternalOutput")
        dbg["QT"] = nc.dram_tensor("dQT", [P, NC * L], BF16,
                                   kind="ExternalOutput")
        dbg["Va"] = nc.dram_tensor("dVa", [P, NT * H * E], FP8,
                                   kind="ExternalOutput")
        dbg["OT0"] = nc.dram_tensor("dOT0", [P, L], BF16,
                                    kind="ExternalOutput")
        dbg["OT7"] = nc.dram_tensor("dOT7", [P, L], BF16,
                                    kind="ExternalOutput")

    with tile.TileContext(nc) as tc:
        with tc.tile_pool(name="persist", bufs=1) as pp:
            eps_t = pp.tile([P, 1], FP32, name="eps_t")
            neg2_t = pp.tile([P, 1], FP32, name="neg2_t")
            bq_t = pp.tile([P, NC], FP32, name="bq_t")
            KT = pp.tile([P, NC, L], BF16, name="KT")
            QT0 = pp.tile([P, 1, L], BF16, name="QT0")
            QTr = pp.tile([P, NC - 1, L], BF16, name="QTr")
            Vaug = pp.tile([P, NT, H * E], FP8, name="Vaug")
            OT = [pp.tile([P, L], BF16, name=f"ot{j}") for j in range(H // 2)]
            qb = [pp.tile([P, D], BF16, name=f"qb{t}") for t in range(NT)]

            nc.gpsimd.dma_start(eps_t, ep_d[:])
            nc.gpsimd.dma_start(neg2_t, n2_d[:])
            nc.gpsimd.dma_start(bq_t, bq_d[:])

            # ---------------- QKV phase (V -> K -> Q) ----------------
            with (
                tc.tile_pool(name="qkv", bufs=1) as qp,
                tc.tile_pool(name="psA", bufs=1, space="PSUM") as psA,
            ):
                def load_w(dram, nm, eng):
                    tiles = []
                    for i in range(NC):
                        wt = qp.tile([P, D], BF16, tag="w", bufs=2 * NC,
                                     name=f"w{nm}{i}")
                        eng.dma_start(wt, dram[i * P:(i + 1) * P, :])
                        tiles.append(wt)
                    return tiles

                def xT_tile():
                    return qp.tile([P, NC, L], BF16, tag="xT", bufs=2,
                                   name="xT")

                # vT first (V-proj is the first PE phase)
                vT = xT_tile()
                for c in range(NC):
                    nc.sync.dma_start(vT[:, c, :], vt_d[c * P:(c + 1) * P, :])
                wv_t = load_w(wv_d, "v", nc.scalar)
                vo1 = qp.tile([P, H * E], FP8, name="vo1")
                nc.gpsimd.dma_start(vo1, vo_d[:])
                for t in range(NT):
                    nc.gpsimd.dma_start(Vaug[:, t, :], vo1)

                kT = xT_tile()
                for c in range(NC):
                    nc.sync.dma_start(kT[:, c, :], kt_d[c * P:(c + 1) * P, :])
                wk_t = load_w(wk_d, "k", nc.scalar)

                # q DMA + LN emitted early; ACT-side LN overlaps V/K-proj.
                # qb stays resident in bf16 and doubles as the residual.
                qn = []
                for t in range(NT):
                    nc.sync.dma_start(qb[t], qb_d[t * P:(t + 1) * P, :])
                    y = qp.tile([P, D], BF16, tag="qn", bufs=NT, name="qn")
                    rstd, nmr = _emit_ln_stats(nc, qp, qb[t], y, eps_t)
                    nc.scalar.activation(y, qb[t], AF.Identity, bias=nmr,
                                         scale=rstd)
                    qn.append(y)
                wq_t = load_w(wq_d, "q", nc.scalar)

                # ---- V-proj (token-major, into Vaug) ----
                for t in range(NT):
                    ps = psA.tile([P, L], FP32, tag="pj", bufs=2, name="ps_v")
                    for n in range(NQ):
                        for i in range(NC):
                            nc.tensor.matmul(
                                ps[:, n * QH:(n + 1) * QH],
                                vT[:, i, t * P:(t + 1) * P],
                                wv_t[i][:, n * QH:(n + 1) * QH],
                                start=(i == 0), stop=(i == NC - 1))
                    dst = Vaug[:, t, :].rearrange("p (h e) -> p h e", e=E)
                    nc.vector.tensor_copy(
                        dst[:, :, 0:HD],
                        ps.rearrange("p (h x) -> p h x", x=HD))

                def proj_feat(w_tiles, src, dst, bias_col=None):
                    dst_of = dst if callable(dst) else (
                        lambda m: dst[:, m, :])
                    for m in range(NC):
                        ps = psA.tile([P, L], FP32, tag="pj", bufs=2,
                                      name="ps_pj")
                        for n in range(NQ):
                            for i in range(NC):
                                nc.tensor.matmul(
                                    ps[:, n * QH:(n + 1) * QH],
                                    w_tiles[i][:, m * P:(m + 1) * P],
                                    src[:, i, n * QH:(n + 1) * QH],
                                    start=(i == 0), stop=(i == NC - 1))
                        if bias_col is None:
                            nc.vector.tensor_copy(dst_of(m), ps)
                        else:
                            nc.vector.tensor_scalar_add(
                                dst_of(m), ps, bias_col[:, m:m + 1])

                # ---- K-proj ----
                proj_feat(wk_t, kT, KT)

                # ---- qn -> qnT via DMA xbar transpose (no PE work) ----
                # xbar transpose needs a per-partition-contiguous dst, so
                # stage [P,NC,P] then scatter into qnT with a plain DMA.
                qnT = xT_tile()
                for t in range(NT):
                    stage = qp.tile([P, NC, P], BF16, tag="tstg", bufs=2,
                                    name="tstg")
                    eng = nc.sync if t % 2 == 0 else nc.scalar
                    eng.dma_start_transpose(stage, qn[t])
                    eng.dma_start(qnT[:, :, t * P:(t + 1) * P], stage)

                # ---- Q-proj (bias folded) ----
                proj_feat(
                    wq_t, qnT,
                    lambda m: QT0[:, 0, :] if m == 0 else QTr[:, m - 1, :],
                    bias_col=bq_t)
                if DEBUG:
                    nc.gpsimd.dma_start(
                        dbg["qnT"][:], qnT.rearrange("p c l -> p (c l)"))
                    nc.gpsimd.dma_start(
                        dbg["KT"][:], KT.rearrange("p c l -> p (c l)"))
                    nc.gpsimd.dma_start(dbg["QT"][:, 0:L], QT0[:, 0, :])
                    nc.gpsimd.dma_start(
                        dbg["QT"][:, L:], QTr.rearrange("p c l -> p (c l)"))
                    nc.gpsimd.dma_start(
                        dbg["Va"][:], Vaug.rearrange("p t e -> p (t e)"))

            # ---------------- out-phase inputs (emit DMAs early) ----------
            with tc.tile_pool(name="fin", bufs=1) as fp:
                gamma_bc = fp.tile([P, D], FP32, name="gamma_bc")
                beta_bc = fp.tile([P, D], FP32, name="beta_bc")
                nc.gpsimd.dma_start(gamma_bc, gb_d[:])
                nc.gpsimd.dma_start(beta_bc, bb_d[:])
                wo_t = []
                for j in range(NC):
                    wt = fp.tile([P, D], BF16, tag="wo", bufs=NC,
                                 name=f"wo{j}")
                    nc.scalar.dma_start(wt, wo_d[j * P:(j + 1) * P, :])
                    wo_t.append(wt)

                # ---------------- attention (paired heads) ----------------
                with (
                    tc.tile_pool(name="att", bufs=1) as ap,
                    tc.tile_pool(name="psS", bufs=1, space="PSUM") as psS,
                    tc.tile_pool(name="psO", bufs=1, space="PSUM") as psO,
                ):
                    rc = RECIP_APPROX_FAST_CONSTS

                    def pv_mms(po, hh, head, PT, i, n):
                        nc.tensor.matmul(
                            po[:, n * QH:(n + 1) * QH],
                            Vaug[:, i, head * E:(head + 1) * E],
                            PT[:, i, hh, n * QH:(n + 1) * QH],
                            start=(i == 0), stop=(i == NT - 1))

                    def epilogue_a(head, po):
                        """Drain one head's PV psum + kick off the
                        reciprocal chain (frees the psO slot fast).

                        The approx-fast DVE reciprocal only works at
                        partition 0, so the sumexp row goes psum(row 64)
                        -> sbuf(lane 64) -> DMA -> sbuf(lane 0)."""
                        rin = fp.tile([E, L], FP32, tag="rin", bufs=2,
                                      name="rin")
                        rec = fp.tile([1, L], FP32R, tag="rec", bufs=2,
                                      name="rec")
                        ou = fp.tile([HD, L], FP32, tag="ou", bufs=2,
                                     name="ou")
                        nc.vector.tensor_copy(rin[HD:E, :], po[HD:E, :])
                        nc.vector.tensor_copy(ou, po[0:HD, :])
                        nc.gpsimd.dma_start(rin[0:1, :], rin[HD:E, :])
                        nc.vector._custom_dve(
                            RECIPROCAL_APPROX_FAST, out=rec, in0=rin[0:1, :],
                            s0=rc["s0"], s1=rc["s1"], imm2=rc["imm2"])
                        return (head, ou, rec)

                    def epilogue_b(head, ou, rec):
                        """Broadcast 1/sumexp across partitions on the idle
                        GpSimd engine, then one DVE multiply into OT."""
                        c2, half = head // 2, head % 2
                        rbc = fp.tile([HD, L], FP32R, tag="rbc", bufs=2,
                                      name="rbc")
                        nc.gpsimd.partition_broadcast(rbc, rec[0:1, :],
                                                      channels=HD)
                        if half == 0:
                            nc.vector.tensor_tensor(
                                OT[c2][0:HD, :], ou, rbc, OP.mult)
                        else:
                            otmp = fp.tile([HD, L], BF16, tag="otmp",
                                           bufs=2, name="otmp")
                            nc.vector.tensor_tensor(otmp, ou, rbc, OP.mult)
                            nc.gpsimd.dma_start(OT[c2][HD:P, 0:QH],
                                                otmp[:, 0:QH])
                            nc.sync.dma_start(OT[c2][HD:P, QH:L],
                                              otmp[:, QH:L])

                    def s_block(c, PT, i, n):
                        """Paired S matmuls for (head 2c, 2c+1), q-half n,
                        key block i, + exp into PT."""
                        ss = psS.tile([P, 2, QH], FP32, tag="s", bufs=2,
                                      name="ps_s")
                        ks = slice(i * P, (i + 1) * P)
                        ns = slice(n * QH, (n + 1) * QH)
                        for hh in range(2):
                            hs = slice(hh * HD, hh * HD + HD)
                            qsrc = (QT0[hs, 0, ns] if c == 0 else
                                    QTr[hs, c - 1, ns])
                            nc.tensor.matmul(ss[:, hh, :], KT[hs, c, ks],
                                             qsrc, start=True, stop=True)
                        # exp(S-3): shift keeps exp within fp8e4m3 range
                        # (overflow needs S>9.1; observed max 7.6+noise);
                        # softmax is shift-invariant and sumexp sums the
                        # shifted values.
                        nc.scalar.activation(PT[:, i, :, ns], ss, AF.Exp,
                                             bias=neg2_t)

                    prev = None   # (c, PT) pending PV
                    epis = []     # pending epilogue_b args
                    for c in range(H // 2):
                        PT = ap.tile([P, NC, 2, L], FP8, tag="pt", bufs=2,
                                     name="pt")
                        po_e = po_o = None
                        if prev is not None:
                            pc, pPT = prev
                            po_e = psO.tile([E, L], FP32, tag="o", bufs=2,
                                            name="ps_oe")
                            po_o = psO.tile([E, L], FP32, tag="o", bufs=2,
                                            name="ps_oo")
                        step = 0
                        for i in range(NT):
                            for n in range(NQ):
                                # PV of the previous pair first (PE queue is
                                # in-order; arriving at the psS-gated S
                                # matmuls later absorbs exp semaphore
                                # latency) -- except at the pair boundary,
                                # where S goes first so the PE has work
                                # while the DVE drains the psO ring.
                                if step < 2:
                                    s_block(c, PT, i, n)
                                if prev is not None:
                                    pv_mms(po_e, 0, 2 * pc, pPT, i, n)
                                    pv_mms(po_o, 1, 2 * pc + 1, pPT, i, n)
                                if step == 5 and epis:
                                    epilogue_b(*epis.pop(0))
                                if step == 9 and epis:
                                    epilogue_b(*epis.pop(0))
                                if step >= 2:
                                    s_block(c, PT, i, n)
                                step += 1
                        if prev is not None:
                            epis.append(epilogue_a(2 * pc, po_e))
                            epis.append(epilogue_a(2 * pc + 1, po_o))
                        prev = (c, PT)

                    # drain: PV of the last pair; its epilogue_b is deferred
                    # into the out-projection warmup so the PE keeps busy
                    # while the reciprocal chains run.
                    while epis:
                        epilogue_b(*epis.pop(0))
                    pc, pPT = prev
                    po_e = psO.tile([E, L], FP32, tag="o", bufs=2,
                                    name="ps_oe")
                    po_o = psO.tile([E, L], FP32, tag="o", bufs=2,
                                    name="ps_oo")
                    for i in range(NT):
                        for n in range(NQ):
                            pv_mms(po_e, 0, 2 * pc, pPT, i, n)
                            pv_mms(po_o, 1, 2 * pc + 1, pPT, i, n)
                    last_epis = [epilogue_a(2 * pc, po_e),
                                 epilogue_a(2 * pc + 1, po_o)]

                if DEBUG:
                    nc.gpsimd.dma_start(dbg["OT0"][:], OT[0][:])
                    nc.gpsimd.dma_start(dbg["OT7"][:], OT[7][:])

                # ------------- output projection + residual + LN ---------
                # Software-pipelined two deep: tile t's j=0..6 matmuls run
                # before tile t-1's j=7 (so the last head pair's OT has
                # time to land), and the LN chain trails by another tile.
                with tc.tile_pool(name="psW", bufs=3, space="PSUM") as psW:
                    def emit_j7_u(t, ps):
                        for n in range(NQ):
                            nc.tensor.matmul(
                                ps[:, n * QH:(n + 1) * QH],
                                OT[NC - 1][:, t * P:(t + 1) * P],
                                wo_t[NC - 1][:, n * QH:(n + 1) * QH],
                                start=False, stop=True)
                        u = fp.tile([P, D], FP32, tag="u", bufs=3, name="u")
                        st = fp.tile([P, 8], FP32, tag="lnst", bufs=3,
                                     name="lnst")
                        nc.vector.scalar_tensor_tensor(
                            u, ps, 0.0, qb[t], OP.add, OP.add,
                            accum_out=st[:, 0:1])
                        return (t, u, st)

                    def emit_ln_out(t, u, st):
                        """Mostly-DVE LN: one cross-engine hop (the tiny
                        sqrt), apply via two STT ops:
                        z = ((u - mu) * gamma) * rstd + beta."""
                        y = fp.tile([P, D], FP32, tag="y", bufs=2, name="y")
                        nc.scalar.activation(y, u, AF.Square,
                                             accum_out=st[:, 1:2])
                        nc.vector.tensor_scalar_mul(st[:, 2:3], st[:, 0:1],
                                                    1.0 / D)
                        nc.vector.tensor_tensor(st[:, 3:4], st[:, 2:3],
                                                st[:, 2:3], OP.mult)
                        nc.vector.tensor_scalar_mul(st[:, 4:5], st[:, 1:2],
                                                    1.0 / D)
                        nc.vector.tensor_tensor(st[:, 4:5], st[:, 4:5],
                                                st[:, 3:4], OP.subtract)
                        nc.scalar.activation(st[:, 5:6], st[:, 4:5],
                                             AF.Sqrt, bias=eps_t)
                        nc.vector.reciprocal(st[:, 6:7], st[:, 5:6])
                        nc.vector.scalar_tensor_tensor(
                            y, u, st[:, 2:3], gamma_bc,
                            OP.subtract, OP.mult)
                        z = fp.tile([P, D], FP32, tag="z", bufs=3, name="z")
                        nc.vector.scalar_tensor_tensor(
                            z, y, st[:, 6:7], beta_bc, OP.mult, OP.add)
                        ts_ = slice(t * P, (t + 1) * P)
                        nc.sync.dma_start(out_d[ts_, 0:QH], z[:, 0:QH])
                        nc.scalar.dma_start(out_d[ts_, QH:D], z[:, QH:D])

                    open_ps = {}
                    lnq = []
                    for t in range(NT):
                        ps = psW.tile([P, D], FP32, tag="w", name="ps_w")
                        for n in range(NQ):
                            for j in range(NC - 1):
                                nc.tensor.matmul(
                                    ps[:, n * QH:(n + 1) * QH],
                                    OT[j][:, t * P:(t + 1) * P],
                                    wo_t[j][:, n * QH:(n + 1) * QH],
                                    start=(j == 0), stop=False)
                        open_ps[t] = ps
                        if t == 1:
                            epilogue_b(*last_epis.pop(0))
                        if t == 2:
                            epilogue_b(*last_epis.pop(0))
                        if t - 1 in open_ps:
                            lnq.append(emit_j7_u(t - 1, open_ps.pop(t - 1)))
                        if len(lnq) > 0:
                            emit_ln_out(*lnq.pop(0))
                    lnq.append(emit_j7_u(NT - 1, open_ps.pop(NT - 1)))
                    for args in lnq:
                        emit_ln_out(*args)

    nc.compile()
    return nc


_CACHE = {}


def _get_nc():
    if "nc" not in _CACHE:
        _CACHE["nc"] = build_bass()
    return _CACHE["nc"]


def make_in_maps(q, k, v, Wq, Wk, Wv, Wo, gamma, beta):
    qb = np.asarray(q, np.float32).astype(ml_dtypes.bfloat16)
    kb = np.asarray(k, np.float32).astype(ml_dtypes.bfloat16)
    vb = np.asarray(v, np.float32).astype(ml_dtypes.bfloat16)
    gamma = np.asarray(gamma, np.float32)
    beta = np.asarray(beta, np.float32)
    Wq = np.asarray(Wq, np.float32)
    # fold pre-LN gamma/beta and the 1/sqrt(dk)=0.125 scale into Wq
    wq = (0.125 * gamma[:, None] * Wq).astype(ml_dtypes.bfloat16)
    bq = (0.125 * (beta @ Wq)).astype(np.float32)           # [D]
    bq_t = np.ascontiguousarray(bq.reshape(NC, P).T)        # [P, NC]
    wk = np.asarray(Wk, np.float32).astype(ml_dtypes.bfloat16)
    wv = np.asarray(Wv, np.float32).astype(ml_dtypes.bfloat16)
    wo = np.asarray(Wo, np.float32).astype(ml_dtypes.bfloat16)
    gb = np.ascontiguousarray(np.tile(gamma[None, :], (P, 1)))
    bb = np.ascontiguousarray(np.tile(beta[None, :], (P, 1)))
    epsc = np.full((P, 1), EPS, np.float32)
    neg2 = np.full((P, 1), -3.0, np.float32)
    vone = np.ones((P, H * E), ml_dtypes.float8_e4m3fn)
    B = q.shape[0]
    return [
        {
            "qb": np.ascontiguousarray(qb[b]),
            "kt": np.ascontiguousarray(kb[b].T),
            "vt": np.ascontiguousarray(vb[b].T),
            "wq": wq, "wk": wk, "wv": wv, "wo": wo, "bq": bq_t,
            "gb": gb, "bb": bb, "epsc": epsc, "vone": vone, "neg2": neg2,
        }
        for b in range(B)
    ]


def kernel(q, k, v, Wq, Wk, Wv, Wo, gamma, beta, trace=False):
    from concourse.bass_utils import run_bass_kernel_spmd

    nc = _get_nc()
    in_maps = make_in_maps(q, k, v, Wq, Wk, Wv, Wo, gamma, beta)
    res = run_bass_kernel_spmd(nc, in_maps, core_ids=list(range(len(in_maps))),
                               trace=trace)
    out = np.stack([r["out"] for r in res.results], axis=0)
    if trace:
        return out, res
    return out
